# revision 17
# baseline (speedup 1.0000x reference)
"""Trainium2 Bass kernel for nn_Cat_Linear_Decoder (GNN edge-MLP decoder).

    out[r] = sigmoid( relu(cat(z[src[r]], z[dst[r]]) @ W1 + b1) @ W2
                      + b2 + sig_bias )        for r in 0..2E  (E = 500k)

Sharding: data-parallel over edge rows across 8 NeuronCores; z + MLP weights
replicated per core.

Device strategy per core:
  - Rows are host-sorted by (src_bank, dst_bank), bank = 32768 nodes, so that
    every dma_gather call reads a single z bank with int16 bank-local indices
    (the SWDGE gather ucode takes int16 indices only).
  - z is stored fp16 in HBM; dma_gather(transpose=True) emits X.T tiles
    [128ch, rows] in SBUF directly, which feed the PE matmuls with the
    contraction (in_ch) on partitions -- no on-chip transposes and half the
    gather bytes vs fp32.
  - Layer 1 computed as two [hid_chunk=128, rows] PSUM tiles:
        h0 = relu(W1[:128,:128].T@Xs + W1[128:,:128].T@Xd + b1[:128])
    relu+bias via ACT for chunk 0 and DVE tensor_scalar for chunk 1.
  - Layer 2: x[1, rows] = W2[:128].T@h0 + W2[128:].T@h1 (PSUM accumulate),
    then ACT sigmoid with folded bias (b2 + sig_bias) into an SBUF strip,
    DMA'd out per gather chunk.
  - Outputs come back in permuted order; the host inverse-permutes.

Host-side work is restricted to sharding/permutation/packing of inputs and
the inverse permutation of the output; all FLOPs of the model run on device.
"""

import os
import sys
from contextlib import ExitStack

import numpy as np

sys.path.insert(0, "/opt/trn_rl_repo")
os.environ.setdefault("MYCRO_LOCAL_CACHE", "1")

import concourse.bacc as bacc
import concourse.mybir as mybir
import concourse.tile as tile
from concourse.bass_utils import run_bass_kernel_spmd

F16 = mybir.dt.float16
F32 = mybir.dt.float32
I16 = mybir.dt.int16

P = 128          # partitions == in_ch per side
HIDDEN = 256
SLICE = 512      # rows per matmul slice (PSUM free-dim limit)
N_CORES = 8
BANK = 32768     # int16-addressable z rows per gather call
GRAN = 128       # group capacity granularity (must be %128)
GCHUNK = 2048    # max rows per dma_gather call

# set by test.py via env to collect a perfetto trace + HW exec time
_TRACE = bool(int(os.environ.get("KERNEL_TRACE", "0")))
last_result = None  # BassKernelResults of the most recent run (for test.py)

_neff_cache = {}


def _make_plan(caps, gchunk):
    plan, off = [], 0
    for bs, bd, cap in caps:
        rem = cap
        while rem > 0:
            n = min(gchunk, rem)
            plan.append((off, n, bs, bd))
            off += n
            rem -= n
    return plan, off


def _build_kernel(v_nodes, bank, plan, m_pad):
    # 4 SWDGE queues: the dma_gather ucode for queue q runs on Q7 core pair
    # {2q, 2q+1}, so spreading gathers across queues parallelizes the
    # descriptor generation (the measured bottleneck) up to 4x.
    # 64KB/partition descriptor rings (4096 descs/queue) so two 2048-idx
    # gathers fit the ring: the Q7 pre-generates gather k+1's descriptors
    # while gather k drains.
    nc = bacc.Bacc(num_swdge_queues=4, dynamic_dma_scratch_size=81920)
    z = nc.dram_tensor("z", [v_nodes, P], F16, kind="ExternalInput")
    si = nc.dram_tensor("si", [P, m_pad // 16], I16, kind="ExternalInput")
    di = nc.dram_tensor("di", [P, m_pad // 16], I16, kind="ExternalInput")
    w1s = nc.dram_tensor("w1s", [P, HIDDEN], F16, kind="ExternalInput")
    w1d = nc.dram_tensor("w1d", [P, HIDDEN], F16, kind="ExternalInput")
    w2 = nc.dram_tensor("w2", [P, 2], F16, kind="ExternalInput")
    b1 = nc.dram_tensor("b1", [P, 3], F32, kind="ExternalInput")
    out = nc.dram_tensor("out", [m_pad], F32, kind="ExternalOutput")

    with tile.TileContext(nc) as tc, ExitStack() as ctx:
        const = ctx.enter_context(tc.tile_pool(name="const", bufs=1))
        si_sb = const.tile([P, m_pad // 16], I16)
        nc.sync.dma_start(si_sb[:], si[:])
        di_sb = const.tile([P, m_pad // 16], I16)
        nc.sync.dma_start(di_sb[:], di[:])
        w1s_sb = const.tile([P, HIDDEN], F16)
        nc.sync.dma_start(w1s_sb[:], w1s[:])
        w1d_sb = const.tile([P, HIDDEN], F16)
        nc.sync.dma_start(w1d_sb[:], w1d[:])
        w2_sb = const.tile([P, 2], F16)
        nc.sync.dma_start(w2_sb[:], w2[:])
        b1_sb = const.tile([P, 3], F32)
        nc.sync.dma_start(b1_sb[:], b1[:])
        ident = const.tile([P, P], F16)
        from concourse.masks import make_identity
        make_identity(nc, ident[:])

        xpool = ctx.enter_context(tc.tile_pool(name="x", bufs=4))
        hpool = ctx.enter_context(tc.tile_pool(name="h", bufs=4))
        opool = ctx.enter_context(tc.tile_pool(name="o", bufs=3))
        pshp = ctx.enter_context(tc.tile_pool(name="pshp", bufs=2, space="PSUM"))
        psxp = ctx.enter_context(tc.tile_pool(name="psxp", bufs=2, space="PSUM"))
        pstp = ctx.enter_context(tc.tile_pool(name="pstp", bufs=2, space="PSUM"))

        qload = [0, 0, 0, 0]

        def pick_queue(n):
            q = min(range(4), key=lambda i: qload[i])
            qload[q] += n
            return q

        for off, n, bs, bd in plan:
            zs = z[bs * bank : min((bs + 1) * bank, v_nodes), :]
            zd = z[bd * bank : min((bd + 1) * bank, v_nodes), :]
            xs = xpool.tile([P, n], F16, tag="xs", name=f"xs_{off}")
            xd = xpool.tile([P, n], F16, tag="xd", name=f"xd_{off}")
            nc.gpsimd.dma_gather(
                out_ap=xs[:].rearrange("p (t e) -> p t e", t=n // P),
                in_ap=zs,
                idxs_ap=si_sb[:, off // 16 : (off + n) // 16],
                num_idxs=n, num_idxs_reg=n, elem_size=P, transpose=False,
                single_packet=False, queue_num=pick_queue(n),
            )
            nc.gpsimd.dma_gather(
                out_ap=xd[:].rearrange("p (t e) -> p t e", t=n // P),
                in_ap=zd,
                idxs_ap=di_sb[:, off // 16 : (off + n) // 16],
                num_idxs=n, num_idxs_reg=n, elem_size=P, transpose=False,
                single_packet=False, queue_num=pick_queue(n),
            )
            ob = opool.tile([1, n], F32, tag="ob", name=f"ob_{off}")
            for j0 in range(0, n, SLICE):
                w = min(SLICE, n - j0)
                sl = slice(j0, j0 + w)
                nb = w // P
                # transpose [row, ch] gather blocks into [ch, row] via PE;
                # src rows land in cols [0, w), dst rows in cols [512, 512+w)
                xt = pstp.tile([P, 2 * SLICE], F16, tag="xt",
                               name=f"xt_{off}_{j0}")
                for b in range(nb):
                    bs_ = slice(j0 + b * P, j0 + (b + 1) * P)
                    nc.tensor.transpose(xt[:, b * P : (b + 1) * P],
                                        xs[:, bs_], ident[:])
                    nc.tensor.transpose(xt[:, SLICE + b * P : SLICE + (b + 1) * P],
                                        xd[:, bs_], ident[:])
                xsT = hpool.tile([P, SLICE], F16, tag="xsT", name=f"xsT_{off}_{j0}")
                xdT = hpool.tile([P, SLICE], F16, tag="xdT", name=f"xdT_{off}_{j0}")
                nc.scalar.copy(xsT[:, :w], xt[:, :w])
                nc.scalar.copy(xdT[:, :w], xt[:, SLICE : SLICE + w])
                hp0 = pshp.tile([P, SLICE], F32, tag="hp0", name=f"hp0_{off}_{j0}")
                hp1 = pshp.tile([P, SLICE], F32, tag="hp1", name=f"hp1_{off}_{j0}")
                nc.tensor.matmul(hp0[:, :w], lhsT=w1s_sb[:, 0:128],
                                 rhs=xsT[:, :w], start=True, stop=False)
                nc.tensor.matmul(hp0[:, :w], lhsT=w1d_sb[:, 0:128],
                                 rhs=xdT[:, :w], start=False, stop=True)
                nc.tensor.matmul(hp1[:, :w], lhsT=w1s_sb[:, 128:256],
                                 rhs=xsT[:, :w], start=True, stop=False)
                nc.tensor.matmul(hp1[:, :w], lhsT=w1d_sb[:, 128:256],
                                 rhs=xdT[:, :w], start=False, stop=True)
                h0 = hpool.tile([P, SLICE], F16, tag="h0", name=f"h0_{off}_{j0}")
                h1 = hpool.tile([P, SLICE], F16, tag="h1", name=f"h1_{off}_{j0}")
                nc.vector.tensor_scalar(out=h0[:, :w], in0=hp0[:, :w],
                                        scalar1=b1_sb[:, 0:1], scalar2=0.0,
                                        op0=mybir.AluOpType.add,
                                        op1=mybir.AluOpType.max)
                nc.vector.tensor_scalar(out=h1[:, :w], in0=hp1[:, :w],
                                        scalar1=b1_sb[:, 1:2], scalar2=0.0,
                                        op0=mybir.AluOpType.add,
                                        op1=mybir.AluOpType.max)
                xp = psxp.tile([1, SLICE], F32, tag="xp", name=f"xp_{off}_{j0}")
                nc.tensor.matmul(xp[:, :w], lhsT=w2_sb[:, 0:1],
                                 rhs=h0[:, :w], start=True, stop=False)
                nc.tensor.matmul(xp[:, :w], lhsT=w2_sb[:, 1:2],
                                 rhs=h1[:, :w], start=False, stop=True)
                nc.scalar.activation(ob[:, sl], xp[:, :w],
                                     mybir.ActivationFunctionType.Sigmoid,
                                     bias=b1_sb[0:1, 2:3], scale=1.0)
            nc.sync.dma_start(out[None, off : off + n], ob[:])
    nc.compile()
    return nc


def _pack_idx16(idx, m_pad):
    """int32 [m_pad] -> int16 [128, m_pad//16] wrapped+replicated layout."""
    t = idx.astype(np.int16).reshape(m_pad // 16, 16).T
    return np.ascontiguousarray(np.tile(t, (8, 1)))


def kernel(z, edge_index, W1, b1, W2, b2, sig_bias):
    global last_result
    z = np.asarray(z)
    edge_index = np.asarray(edge_index)
    W1 = np.asarray(W1, dtype=np.float32)
    b1 = np.asarray(b1, dtype=np.float32)
    W2 = np.asarray(W2, dtype=np.float32)
    b2 = np.asarray(b2, dtype=np.float32)
    sig_bias = np.asarray(sig_bias, dtype=np.float32)

    v = z.shape[0]
    e = edge_index.shape[1]
    r = 2 * e
    per = r // N_CORES
    nb = (v + BANK - 1) // BANK

    src = np.concatenate([edge_index[0], edge_index[1]]).astype(np.int32)
    dst = np.concatenate([edge_index[1], edge_index[0]]).astype(np.int32)

    # per-core grouping by (src_bank, dst_bank)
    per_core = []
    counts_all = np.zeros((N_CORES, nb * nb), dtype=np.int64)
    for c in range(N_CORES):
        s = src[c * per : (c + 1) * per]
        d = dst[c * per : (c + 1) * per]
        gid = (s // BANK) * nb + (d // BANK)
        order = np.argsort(gid, kind="stable")
        counts = np.bincount(gid, minlength=nb * nb)
        counts_all[c] = counts
        per_core.append((s, d, order, counts))

    maxc = counts_all.max(axis=0)
    caps = []
    for g in range(nb * nb):
        if maxc[g] == 0:
            continue
        caps.append((g // nb, g % nb, int(-(-maxc[g] // GRAN) * GRAN)))
    plan, m_pad = _make_plan(caps, GCHUNK)

    zf = np.ascontiguousarray(z.astype(np.float16))
    w1s = np.ascontiguousarray(W1[:P, :].astype(np.float16))
    w1d = np.ascontiguousarray(W1[P:, :].astype(np.float16))
    w2p = np.ascontiguousarray(
        np.stack([W2[:P, 0], W2[P:, 0]], axis=1).astype(np.float16))
    bias2 = float(np.float32(b2[0]) + np.float32(sig_bias[0]))
    b1p = np.ascontiguousarray(
        np.stack([b1[:P], b1[P:], np.full(P, bias2)], axis=1).astype(np.float32))

    in_maps = []
    orig_rows = []
    for c in range(N_CORES):
        s, d, order, counts = per_core[c]
        sp = np.zeros(m_pad, dtype=np.int32)
        dp = np.zeros(m_pad, dtype=np.int32)
        og = np.full(m_pad, -1, dtype=np.int64)
        cum = np.concatenate([[0], np.cumsum(counts)])
        off = 0
        for bs, bd, cap in caps:
            g = bs * nb + bd
            cnt = int(counts[g])
            rows = order[cum[g] : cum[g] + cnt]
            sp[off : off + cnt] = s[rows] - bs * BANK
            dp[off : off + cnt] = d[rows] - bd * BANK
            og[off : off + cnt] = c * per + rows
            off += cap
        in_maps.append({
            "z": zf,
            "si": _pack_idx16(sp, m_pad),
            "di": _pack_idx16(dp, m_pad),
            "w1s": w1s, "w1d": w1d, "w2": w2p, "b1": b1p,
        })
        orig_rows.append(og)

    key = (v, m_pad, tuple(plan))
    if key not in _neff_cache:
        _neff_cache[key] = _build_kernel(v, BANK, plan, m_pad)
    nc = _neff_cache[key]

    res = run_bass_kernel_spmd(nc, in_maps, list(range(N_CORES)), trace=_TRACE)
    last_result = res

    result = np.zeros(r, dtype=np.float32)
    for o, og in zip(res.results, orig_rows):
        m = og >= 0
        result[og[m]] = np.asarray(o["out"], dtype=np.float32)[m]
    return result
